# revision 8
# baseline (speedup 1.0000x reference)
"""Trainium2 Bass kernel for NeighborhoodNormalization.

Math: the reference builds a per-point homogeneous transform
T = [[ux,-uy,0,px],[uy,ux,0,py],[0,0,1,pz],[0,0,0,1]] (u = p/||p||),
inverts it, and applies it to 64 neighbors per point.  Closed form with
r2 = px^2+py^2, n = ||p||, a = n/r2, cx = px*a, cy = py*a, d = q - p:

    out.x =  cx*dx + cy*dy
    out.y = -cy*dx + cx*dy
    out.z =  dz

Strategy (memory-bound, tolerance 2e-2 allows bf16):
  * Host converts neighborhoods to bf16 in the exact per-core SBUF layout
    [128 partitions, 128 cols, 3, 64] and upcasts/reshapes the bf16 output
    -> halves HBM traffic, and every group DMA is one contiguous 6 KB run
    per partition (near line-rate descriptors; plain HWDGE both directions).
    (DMA-accumulate was tried for d = q - p and abandoned: CCE descriptors
    run at ~half rate and silently stop accumulating past 2048 elems/run.)
  * All elementwise math runs as wide step-1 bf16 tensor_tensor ops.
    Per-point values (coefficients, -p) enter as pair-duplicated [P,T,2]
    bf16 tiles viewed with (..)(K/2:0)(2:1) broadcast APs so the 16-bit 2x
    DVE perf mode still applies (confirmed ~631 ns/1024 elems on HW).
  * x/y planes are processed as merged [P,G,2,K] ops: d_xy = q_xy - p_xy,
    m14 = d_xy*cx (= m1,m4), m23 = d_xy*cy (= m3,m2), then
    ox = m14[0]+m23[1], oy = m14[1]-m23[0] overwrite q_xy in the input tile
    so one DMA per group moves all three output planes out.
  * Load balance: DVE does the 4 m/o passes every group; the z subtraction
    runs on GpSimd (in place); the xy subtraction alternates DVE/GpSimd.

Sharding: pure data parallel over N=8192 points across 8 cores.
Per-core layout: 16384 points = 128 partitions x 128 columns, partition
p = b*8 + s holds points with local n = s*128 + t.
"""

import sys

if "/opt/trn_rl_repo" not in sys.path:
    sys.path.insert(0, "/opt/trn_rl_repo")

import numpy as np
from ml_dtypes import bfloat16

import concourse.bass as bass
import concourse.bacc as bacc
import concourse.mybir as mybir
from concourse.tile import TileContext
from concourse.bass_utils import run_bass_kernel_spmd

B = 16
N = 8192
K = 64
NCORES = 8
NLOC = N // NCORES  # 1024 points per core
P = 128             # SBUF partitions
S = NLOC // P       # 8 partition sub-blocks per batch entry
T = (B * NLOC) // P  # 128 point-columns per partition
G = 16              # columns per group
NG = T // G

F32 = mybir.dt.float32
BF16 = mybir.dt.bfloat16
OP = mybir.AluOpType
AF = mybir.ActivationFunctionType

_CACHE = {}


def _build_nc():
    nc = bacc.Bacc(None, target_bir_lowering=False)

    pts = nc.declare_dram_parameter("points", [B, NLOC, 3], F32, isOutput=False)
    nb = nc.declare_dram_parameter("nbh", [P, T, 3, K], BF16, isOutput=False)
    out = nc.declare_dram_parameter("outh", [P, T, 3, K], BF16, isOutput=True)

    nbr = nb[:]
    outr = out[:]
    ptsr = pts[:].rearrange("b (s t) c -> (b s) (t c)", s=S)

    with TileContext(nc) as tc:
        with tc.tile_pool(name="const", bufs=1) as cpool, \
             tc.tile_pool(name="io", bufs=1) as iopool, \
             tc.tile_pool(name="tmp", bufs=3) as tmppool:

            pts_sb = cpool.tile([P, T * 3], F32, tag="pts")
            nc.sync.dma_start(out=pts_sb[:], in_=ptsr)
            pv = pts_sb[:].rearrange("p (t c) -> p t c", c=3)
            px = pv[:, :, 0]
            py = pv[:, :, 1]
            pz = pv[:, :, 2]

            def ctile(tag, dtype=F32, shape=None):
                return cpool.tile(shape or [P, T], dtype, tag=tag, name=tag)

            # pair-duplicated negated points (only need pts): emit first so
            # they are ready before the coefficient chain finishes.
            nxyd = ctile("nxyd", BF16, [P, T, 2, 2])   # (-px,-px),(-py,-py)
            nzd = ctile("nzd", BF16, [P, T, 2])        # (-pz,-pz)
            nc.vector.tensor_scalar_mul(
                out=nxyd[:], in0=pv[:, :, 0:2, None].broadcast_to([P, T, 2, 2]),
                scalar1=-1.0,
            )
            nc.vector.tensor_scalar_mul(
                out=nzd[:], in0=pv[:, :, 2, None].broadcast_to([P, T, 2]),
                scalar1=-1.0,
            )

            t1 = ctile("t1")
            t2 = ctile("t2")
            r2 = ctile("r2")
            n2 = ctile("n2")
            nn = ctile("nn")
            ir2 = ctile("ir2")
            aa = ctile("aa")
            cx = ctile("cx")
            cy = ctile("cy")

            nc.vector.tensor_mul(out=t1[:], in0=px, in1=px)
            nc.vector.tensor_mul(out=t2[:], in0=py, in1=py)
            nc.vector.tensor_add(out=r2[:], in0=t1[:], in1=t2[:])
            nc.vector.tensor_mul(out=t1[:], in0=pz, in1=pz)
            nc.vector.tensor_add(out=n2[:], in0=r2[:], in1=t1[:])
            nc.scalar.sqrt(out=nn[:], in_=n2[:])
            nc.vector.reciprocal(out=ir2[:], in_=r2[:])
            nc.vector.tensor_mul(out=aa[:], in0=nn[:], in1=ir2[:])
            nc.vector.tensor_mul(out=cx[:], in0=px, in1=aa[:])
            nc.vector.tensor_mul(out=cy[:], in0=py, in1=aa[:])

            # pair-duplicated bf16 coefficient tiles: cxd[p,t,:] = (cx, cx)
            cxd = ctile("cxd", BF16, [P, T, 2])
            cyd = ctile("cyd", BF16, [P, T, 2])
            nc.vector.tensor_copy(cxd[:], cx[:, :, None].broadcast_to([P, T, 2]))
            nc.vector.tensor_copy(cyd[:], cy[:, :, None].broadcast_to([P, T, 2]))

            def pair(ap):
                # [P, G, K] -> [P, G, K/2, 2] (innermost step-1 pair)
                return ap.rearrange("p g (h two) -> p g h two", two=2)

            for g in range(NG):
                g0, g1 = g * G, (g + 1) * G
                sc = iopool.tile([P, G, 3, K], BF16, tag=f"sc{g}", name=f"sc{g}")
                nc.sync.dma_start(out=sc[:], in_=nbr[:, g0:g1])

                qx = pair(sc[:, :, 0, :])
                qy = pair(sc[:, :, 1, :])
                qz = pair(sc[:, :, 2, :])
                SHP = [P, G, K // 2, 2]
                nx_b = nxyd[:, g0:g1, 0, None, :].broadcast_to(SHP)
                ny_b = nxyd[:, g0:g1, 1, None, :].broadcast_to(SHP)
                nz_b = nzd[:, g0:g1, None, :].broadcast_to(SHP)

                def mtile(tag):
                    return tmppool.tile(SHP, BF16, tag=tag, name=f"{tag}_{g}")

                dx = mtile("dx")
                dy = mtile("dy")
                nc.vector.tensor_add(out=dx[:], in0=qx, in1=nx_b)
                nc.vector.tensor_add(out=dy[:], in0=qy, in1=ny_b)
                # z in place on GpSimd: oz = qz - pz
                nc.gpsimd.tensor_add(out=qz, in0=qz, in1=nz_b)

                bcx = cxd[:, g0:g1, None, :].broadcast_to(SHP)
                bcy = cyd[:, g0:g1, None, :].broadcast_to(SHP)
                m1 = mtile("m1")
                m2 = mtile("m2")
                m3 = mtile("m3")
                m4 = mtile("m4")
                nc.vector.tensor_mul(out=m1[:], in0=dx[:], in1=bcx)
                nc.vector.tensor_mul(out=m2[:], in0=dy[:], in1=bcy)
                nc.vector.tensor_mul(out=m3[:], in0=dx[:], in1=bcy)
                nc.vector.tensor_mul(out=m4[:], in0=dy[:], in1=bcx)
                nc.vector.tensor_add(out=qx, in0=m1[:], in1=m2[:])
                nc.vector.tensor_sub(out=qy, in0=m4[:], in1=m3[:])

                nc.scalar.dma_start(out=outr[:, g0:g1], in_=sc[:])

    nc.compile()
    return nc


def _get_nc():
    if "nc" not in _CACHE:
        _CACHE["nc"] = _build_nc()
    return _CACHE["nc"]


def make_in_maps(points, neighborhoods):
    pts = np.ascontiguousarray(np.asarray(points, dtype=np.float32))
    nb = np.asarray(neighborhoods, dtype=np.float32)
    assert pts.shape == (B, N, 3), pts.shape
    assert nb.shape == (B, N, K, 3), nb.shape

    nb16 = nb.astype(bfloat16)  # [B, N, K, 3]

    in_maps = []
    for c in range(NCORES):
        sl = slice(c * NLOC, (c + 1) * NLOC)
        # [B, NLOC, K, 3] -> [B, S, T, 3, K] -> [P, T, 3, K]
        nbh = (
            nb16[:, sl]
            .reshape(B, S, T, K, 3)
            .transpose(0, 1, 2, 4, 3)
            .reshape(P, T, 3, K)
        )
        in_maps.append({
            "points": np.ascontiguousarray(pts[:, sl]),
            "nbh": np.ascontiguousarray(nbh),
        })
    return in_maps


def kernel(points, neighborhoods):
    in_maps = make_in_maps(points, neighborhoods)
    res = run_bass_kernel_spmd(_get_nc(), in_maps, list(range(NCORES))).results
    # [P, T, 3, K] -> [B, S, T, K, 3] -> [B, NLOC, K, 3], then concat cores
    parts = []
    for c in range(NCORES):
        o = np.asarray(res[c]["outh"]).reshape(B, S, T, 3, K)
        parts.append(o.transpose(0, 1, 2, 4, 3).reshape(B, NLOC, K, 3))
    return np.concatenate(parts, axis=1).astype(np.float32)


# revision 9
# speedup vs baseline: 1.2776x; 1.2776x over previous
"""Trainium2 Bass kernel for NeighborhoodNormalization.

Math: the reference builds a per-point homogeneous transform
T = [[ux,-uy,0,px],[uy,ux,0,py],[0,0,1,pz],[0,0,0,1]] (u = p/||p||),
inverts it, and applies it to 64 neighbors per point.  Closed form with
r2 = px^2+py^2, n = ||p||, a = n/r2, cx = px*a, cy = py*a, d = q - p:

    out.x =  cx*dx + cy*dy
    out.y = -cy*dx + cx*dy   (= dy*cx + dx*(-cy))
    out.z =  dz

Strategy (memory-bound, tolerance 2e-2 allows bf16):
  * Host converts neighborhoods to bf16 in the exact per-core SBUF layout
    [128 partitions, 128 cols, 3, 64] and upcasts/reshapes the bf16 output
    -> halves HBM traffic; every group DMA is one contiguous 6 KB run per
    partition (415 GB/s measured on the input stream).
  * Elementwise math runs as wide step-1 bf16 tensor_tensor ops on DVE.
    Per-point values (coefficients, -p) enter as pair-duplicated [P,T,2]
    bf16 tiles viewed with (..)(K/2:0)(2:1) broadcast APs so the 16-bit 2x
    DVE perf mode applies (measured ~677 ns / 1024-elem op).
  * DVE does 7 ops/group: dx, dy, z (in place), m1 = dx*cx, m2 = dy*cy,
    m3n = dx*(-cy), m4 = dy*cx.  The final sums run on the otherwise-idle
    TensorE as identity-matmul PSUM accumulations (ox = m1+m2, oy = m4+m3n)
    and ACT copies PSUM back into the x/y planes of the group tile, so one
    DMA per group moves all three output planes out.
  * GpSimd does NOTHING: its elementwise ops share an SBUF port with DVE
    and degrade DVE throughput ~3.5x (measured); DMA-accumulate via SWDGE
    was also abandoned (CCE descriptors run at ~half DMA rate and silently
    stop accumulating past 2048 elems/run).

Sharding: pure data parallel over N=8192 points across 8 cores.
Per-core layout: 16384 points = 128 partitions x 128 columns, partition
p = b*8 + s holds points with local n = s*128 + t.
"""

import sys

if "/opt/trn_rl_repo" not in sys.path:
    sys.path.insert(0, "/opt/trn_rl_repo")

import numpy as np
from ml_dtypes import bfloat16

import concourse.bass as bass
import concourse.bacc as bacc
import concourse.mybir as mybir
from concourse.bass import MemorySpace
from concourse.tile import TileContext
from concourse.bass_utils import run_bass_kernel_spmd

B = 16
N = 8192
K = 64
NCORES = 8
NLOC = N // NCORES  # 1024 points per core
P = 128             # SBUF partitions
S = NLOC // P       # 8 partition sub-blocks per batch entry
T = (B * NLOC) // P  # 128 point-columns per partition
G = 16              # columns per group
NG = T // G
GK = G * K          # 1024 elems per (group, plane)
MM = 512            # moving free-dim max per matmul

F32 = mybir.dt.float32
BF16 = mybir.dt.bfloat16
OP = mybir.AluOpType
AF = mybir.ActivationFunctionType

_CACHE = {}


def _build_nc():
    nc = bacc.Bacc(None, target_bir_lowering=False)

    pts = nc.declare_dram_parameter("points", [B, NLOC, 3], F32, isOutput=False)
    nb = nc.declare_dram_parameter("nbh", [P, T, 3, K], BF16, isOutput=False)
    ident = nc.declare_dram_parameter("ident", [P, P], BF16, isOutput=False)
    out = nc.declare_dram_parameter("outh", [P, T, 3, K], BF16, isOutput=True)

    nbr = nb[:]
    outr = out[:]
    ptsr = pts[:].rearrange("b (s t) c -> (b s) (t c)", s=S)

    with TileContext(nc) as tc:
        with tc.tile_pool(name="const", bufs=1) as cpool, \
             tc.tile_pool(name="io", bufs=1) as iopool, \
             tc.tile_pool(name="tmp", bufs=3) as tmppool, \
             tc.tile_pool(name="ps", bufs=2, space=MemorySpace.PSUM) as pspool:

            pts_sb = cpool.tile([P, T * 3], F32, tag="pts")
            nc.sync.dma_start(out=pts_sb[:], in_=ptsr)
            id_sb = cpool.tile([P, P], BF16, tag="id")
            nc.sync.dma_start(out=id_sb[:], in_=ident[:])

            pv = pts_sb[:].rearrange("p (t c) -> p t c", c=3)
            px = pv[:, :, 0]
            py = pv[:, :, 1]
            pz = pv[:, :, 2]

            def ctile(tag, dtype=F32, shape=None):
                return cpool.tile(shape or [P, T], dtype, tag=tag, name=tag)

            # pair-duplicated negated points (need only pts): emitted first
            nxyd = ctile("nxyd", BF16, [P, T, 2, 2])   # (-px,-px),(-py,-py)
            nzd = ctile("nzd", BF16, [P, T, 2])        # (-pz,-pz)
            nc.vector.tensor_scalar_mul(
                out=nxyd[:], in0=pv[:, :, 0:2, None].broadcast_to([P, T, 2, 2]),
                scalar1=-1.0,
            )
            nc.vector.tensor_scalar_mul(
                out=nzd[:], in0=pv[:, :, 2, None].broadcast_to([P, T, 2]),
                scalar1=-1.0,
            )

            t1 = ctile("t1")
            t2 = ctile("t2")
            r2 = ctile("r2")
            n2 = ctile("n2")
            nn = ctile("nn")
            ir2 = ctile("ir2")
            aa = ctile("aa")
            cx = ctile("cx")
            cy = ctile("cy")

            nc.vector.tensor_mul(out=t1[:], in0=px, in1=px)
            nc.vector.tensor_mul(out=t2[:], in0=py, in1=py)
            nc.vector.tensor_add(out=r2[:], in0=t1[:], in1=t2[:])
            nc.vector.tensor_mul(out=t1[:], in0=pz, in1=pz)
            nc.vector.tensor_add(out=n2[:], in0=r2[:], in1=t1[:])
            nc.scalar.sqrt(out=nn[:], in_=n2[:])
            nc.vector.reciprocal(out=ir2[:], in_=r2[:])
            nc.vector.tensor_mul(out=aa[:], in0=nn[:], in1=ir2[:])
            nc.vector.tensor_mul(out=cx[:], in0=px, in1=aa[:])
            nc.vector.tensor_mul(out=cy[:], in0=py, in1=aa[:])

            # pair-duplicated bf16 coefficient tiles
            cxd = ctile("cxd", BF16, [P, T, 2])    # (cx, cx)
            cyd = ctile("cyd", BF16, [P, T, 2])    # (cy, cy)
            ncyd = ctile("ncyd", BF16, [P, T, 2])  # (-cy, -cy)
            nc.vector.tensor_copy(cxd[:], cx[:, :, None].broadcast_to([P, T, 2]))
            nc.vector.tensor_copy(cyd[:], cy[:, :, None].broadcast_to([P, T, 2]))
            nc.vector.tensor_scalar_mul(
                out=ncyd[:], in0=cy[:, :, None].broadcast_to([P, T, 2]),
                scalar1=-1.0,
            )

            def pair(ap):
                # [P, G, K] -> [P, G, K/2, 2] (innermost step-1 pair)
                return ap.rearrange("p g (h two) -> p g h two", two=2)

            for g in range(NG):
                g0, g1 = g * G, (g + 1) * G
                sc = iopool.tile([P, G, 3, K], BF16, tag=f"sc{g}", name=f"sc{g}")
                nc.sync.dma_start(out=sc[:], in_=nbr[:, g0:g1])

                qx = pair(sc[:, :, 0, :])
                qy = pair(sc[:, :, 1, :])
                qz = pair(sc[:, :, 2, :])
                SHP = [P, G, K // 2, 2]
                nx_b = nxyd[:, g0:g1, 0, None, :].broadcast_to(SHP)
                ny_b = nxyd[:, g0:g1, 1, None, :].broadcast_to(SHP)
                nz_b = nzd[:, g0:g1, None, :].broadcast_to(SHP)

                def mtile(tag):
                    # flat [P, GK] so matmul chunks slice contiguously
                    return tmppool.tile([P, GK], BF16, tag=tag, name=f"{tag}_{g}")

                dx = mtile("dx")
                dy = mtile("dy")
                dx4 = pair(dx[:].rearrange("p (g k) -> p g k", g=G))
                dy4 = pair(dy[:].rearrange("p (g k) -> p g k", g=G))
                nc.vector.tensor_add(out=dx4, in0=qx, in1=nx_b)
                nc.vector.tensor_add(out=dy4, in0=qy, in1=ny_b)
                # oz = qz - pz in place on DVE (same-engine program order)
                nc.vector.tensor_add(out=qz, in0=qz, in1=nz_b)

                bcx = cxd[:, g0:g1, None, :].broadcast_to(SHP)
                bcy = cyd[:, g0:g1, None, :].broadcast_to(SHP)
                bncy = ncyd[:, g0:g1, None, :].broadcast_to(SHP)
                m1 = mtile("m1")    # dx*cx
                m2 = mtile("m2")    # dy*cy
                m3n = mtile("m3n")  # dx*(-cy)
                m4 = mtile("m4")    # dy*cx
                for mt, din, cf in ((m1, dx4, bcx), (m2, dy4, bcy),
                                    (m3n, dx4, bncy), (m4, dy4, bcx)):
                    nc.vector.tensor_mul(
                        out=pair(mt[:].rearrange("p (g k) -> p g k", g=G)),
                        in0=din, in1=cf,
                    )

                # ox = m1 + m2, oy = m4 + m3n on TensorE (identity matmul,
                # PSUM fp32 accumulation), ACT copies back into sc
                psx = pspool.tile([P, GK], F32, tag="psx", name=f"psx{g}")
                psy = pspool.tile([P, GK], F32, tag="psy", name=f"psy{g}")
                for ps, ma, mb in ((psx, m1, m2), (psy, m4, m3n)):
                    for c0 in range(0, GK, MM):
                        nc.tensor.matmul(
                            ps[:, c0:c0 + MM], id_sb[:], ma[:, c0:c0 + MM],
                            start=True, stop=False,
                        )
                        nc.tensor.matmul(
                            ps[:, c0:c0 + MM], id_sb[:], mb[:, c0:c0 + MM],
                            start=False, stop=True,
                        )
                nc.scalar.activation(
                    out=sc[:, :, 0, :],
                    in_=psx[:].rearrange("p (g k) -> p g k", g=G),
                    func=AF.Copy,
                )
                nc.scalar.activation(
                    out=sc[:, :, 1, :],
                    in_=psy[:].rearrange("p (g k) -> p g k", g=G),
                    func=AF.Copy,
                )

                nc.scalar.dma_start(out=outr[:, g0:g1], in_=sc[:])

    nc.compile()
    return nc


def _get_nc():
    if "nc" not in _CACHE:
        _CACHE["nc"] = _build_nc()
    return _CACHE["nc"]


def make_in_maps(points, neighborhoods):
    pts = np.ascontiguousarray(np.asarray(points, dtype=np.float32))
    nb = np.asarray(neighborhoods, dtype=np.float32)
    assert pts.shape == (B, N, 3), pts.shape
    assert nb.shape == (B, N, K, 3), nb.shape

    nb16 = nb.astype(bfloat16)  # [B, N, K, 3]
    ident = np.eye(P, dtype=bfloat16)

    in_maps = []
    for c in range(NCORES):
        sl = slice(c * NLOC, (c + 1) * NLOC)
        # [B, NLOC, K, 3] -> [B, S, T, 3, K] -> [P, T, 3, K]
        nbh = (
            nb16[:, sl]
            .reshape(B, S, T, K, 3)
            .transpose(0, 1, 2, 4, 3)
            .reshape(P, T, 3, K)
        )
        in_maps.append({
            "points": np.ascontiguousarray(pts[:, sl]),
            "nbh": np.ascontiguousarray(nbh),
            "ident": ident,
        })
    return in_maps


def kernel(points, neighborhoods):
    in_maps = make_in_maps(points, neighborhoods)
    res = run_bass_kernel_spmd(_get_nc(), in_maps, list(range(NCORES))).results
    # [P, T, 3, K] -> [B, S, T, K, 3] -> [B, NLOC, K, 3], then concat cores
    parts = []
    for c in range(NCORES):
        o = np.asarray(res[c]["outh"]).reshape(B, S, T, 3, K)
        parts.append(o.transpose(0, 1, 2, 4, 3).reshape(B, NLOC, K, 3))
    return np.concatenate(parts, axis=1).astype(np.float32)
